# revision 1
# baseline (speedup 1.0000x reference)
"""Trainium2 Bass kernel for nn_DiscriminationLoss (segment_reduce).

v2 design (8 NeuronCores, pixel-sharded; full inputs in, full loss out):

  - Each core gets 1/8 of the 4M pixels: pred slice [8, 524288] f32 and
    labels slice [524288] i32, tiled [128 partitions x 4096 block-cols].
  - One-hot generation on DVE via per-class tensor_scalar(is_equal, j)
    at 4x perf mode (16-bit dense step-1 APs): 32 ops per 1024-col chunk,
    ~42us/core total (vs ~70us for the tensor_tensor+iota variant: TT is
    capped at 2x; TS reaches 4x_2p).  Each TS op also emits accum_out =
    per-partition sum of its one-hot = per-chunk label counts, so no
    "ones" column is needed in the matmul stationary.
  - pred is scaled by 2^14 and cast f32->fp16 on ScalarE into the
    block-diagonal stationary layout [p, (unit, c, b)] (qb=8 blocks per
    unit, 8 channels -> 64-column stationary slabs).
  - The PE runs TWO concurrent column-tiles (128x64 array mode):
    tile t in {0,1} handles units u with u%2==t,
      psum[64t:64t+64, :256] += slabh_unit[128, 64]^T @ oh_unit[128, 256]
    Each tile streams its own moving operand, so the aggregate moving
    rate is ~2 cols/cycle: ~28us of PE vs ~56us untiled.  Only the 8
    diagonal [8, 32] sub-blocks of each [64, 256] product are meaningful;
    the host extracts them.
  - Labels ride the hardware DMA queue (Sync) as int32, interleaved
    ahead of the pred groups, and are cast int32->bf16 on ScalarE.  (The
    SWDGE cast-DMA used previously is a hidden ~50us serial stream.)
  - Per-kernel counts ride the stationary's 9th (ones) column — the
    36-col stationary leaves room in the 64-col tile, so no accum_out
    (whose CACHE_REDUCE lowering runs at 1x) and no extra streams.
  - Warmup matmuls on a memset tile trip the PE HAM clock gate during
    the otherwise-dead first ~12us (DMA/one-hot pipeline fill).
  - Host sums per-core partials (the "psum" step of the sharding hint)
    and evaluates the tiny O(K^2) pairwise tail in f64.
"""

import sys
import functools

sys.path.insert(0, "/opt/trn_rl_repo")

import numpy as np

C = 8
K = 32
NCORES = 8
H = W = 2048
PTOT = H * W
PCORE = PTOT // NCORES  # 524288
SIGMA_DIS = 3.0
PRED_SCALE = float(2.0**14)

QB = 4            # pixel-blocks per matmul unit (block-diagonal trick)
NCH = C + 1       # 8 pred channels + ones column (counts)
NSTAT = NCH * QB  # stationary columns per unit = 36 (fits the 64-col tile)
NMOV = K * QB     # moving columns per unit = 128
WARM_MMS = 96     # PE warmup matmuls (trip the HAM clock gate early)


def _group_sizes(ftot, fg):
    """Pred DMA/cast group sizes (block-cols): small head groups prime the
    pipeline, small tail groups shorten the exposed DMA tail."""
    if ftot >= 8 * fg:
        gs = (
            [fg // 4, 3 * fg // 4]
            + [fg] * (ftot // fg - 2)
            + [fg // 2, fg // 4, fg // 4]
        )
    else:
        gs = [fg] * (ftot // fg)
    assert sum(gs) == ftot
    return gs


def _chunk_sizes(ftot, fc):
    if ftot <= fc:
        return [ftot]
    if ftot >= 4 * fc:
        return [fc // 2] + [fc] * (ftot // fc - 1) + [fc // 2]
    return [fc] * (ftot // fc)


def build_nc(pcore=PCORE, fg=256, fc=1024, warm=WARM_MMS):
    import concourse.bacc as bacc
    import concourse.tile as tile
    import concourse.mybir as mybir
    from contextlib import ExitStack

    assert pcore % 128 == 0
    ftot = pcore // 128
    fg = min(fg, ftot)
    fc = min(fc, ftot)
    assert ftot % fg == 0 and ftot % fc == 0
    gs = _group_sizes(ftot, fg)
    cs = _chunk_sizes(ftot, fc)
    nchunks = len(cs)
    nunits = ftot // QB

    f32 = mybir.dt.float32
    bf16 = mybir.dt.bfloat16
    fp16 = mybir.dt.float16
    i32 = mybir.dt.int32

    nc = bacc.Bacc(
        "TRN2", target_bir_lowering=False, debug=False, num_devices=NCORES
    )
    # Host pre-packs pixel-major layouts: pred [p, (f, c)] and labels [p, f]
    # (pixel (p, t) = core_linear[p*ftot + t]).  Every group DMA is then a
    # dense 2D copy with 16KB-per-partition-row runs — big descriptors are
    # the difference between ~300 GB/s (2KB) and ~350+ GB/s effective.
    pred_ext = nc.dram_tensor("pred", [128, ftot * C], f32, kind="ExternalInput")
    lab_ext = nc.dram_tensor("labels", [128, ftot], i32, kind="ExternalInput")
    out_ext = nc.dram_tensor("out_s", [128, NMOV], f32, kind="ExternalOutput")
    # row 32: warm dump (keeps warm MMs live)
    aux_ext = nc.dram_tensor("out_aux", [40, 128], f32, kind="ExternalOutput")

    with tile.TileContext(nc) as tc, ExitStack() as ctx:
        const_pool = ctx.enter_context(tc.tile_pool(name="const", bufs=1))
        labbf_pool = ctx.enter_context(tc.tile_pool(name="labbf", bufs=1))
        slabh_pool = ctx.enter_context(tc.tile_pool(name="slabh", bufs=4))
        slab32_pool = ctx.enter_context(tc.tile_pool(name="slab32", bufs=4))
        oh_pool = ctx.enter_context(tc.tile_pool(name="oh", bufs=2))
        psum_pool = ctx.enter_context(tc.tile_pool(name="psum", bufs=1, space="PSUM"))
        out_pool = ctx.enter_context(tc.tile_pool(name="outp", bufs=1))

        # constants / scratch
        warm_t = const_pool.tile([128, 128], bf16)
        nc.gpsimd.memset(warm_t[:], 1.0)
        labbf = labbf_pool.tile([128, ftot], bf16)
        lab32 = labbf_pool.tile([128, ftot], i32)
        # Two static slabh buffers, alternated per group.  The ones slots
        # (c == 8 of every (u, b)) are memset once and never overwritten —
        # pred casts only touch c < 8 — so no per-group ones pass is needed
        # (strided ones writes measured ~1.45us/group on ACT).
        slabh_bufs = []
        for _ in range(4):
            sb = slabh_pool.tile([128, NSTAT * (fg // QB)], fp16, tag="slabh")
            sbr = sb.rearrange("p (u b c) -> p u b c", c=NCH, b=QB)
            nc.vector.memset(sbr[:, :, :, C], 1.0)
            slabh_bufs.append(sb)
        outaux = out_pool.tile([128, 128], f32)
        nc.gpsimd.memset(outaux[:40, :], 0.0)

        psum_s = psum_pool.tile([128, NMOV], f32)
        warm_ps = psum_pool.tile([128, 128], f32)

        # PE warmup on memset data: runs during the head DMA/one-hot fill.
        if warm:
            for w in range(warm):
                nc.tensor.matmul(
                    warm_ps[:64, :],
                    warm_t[:, :64],
                    warm_t[:, :128],
                    start=(w == 0),
                    stop=(w == warm - 1),
                )

        # ---- DMA + cast streams -------------------------------------------
        # Per-group p-major pixel mapping (pixel (p, t) = group_lin[p*fgg+f])
        # shared by the label and pred streams — keeps every DMA a fully
        # CONTIGUOUS HBM read (scattered 2KB runs measured ~284 GB/s vs
        # ~340 contiguous).  Labels interleave with pred on the sync queue
        # into one static i32 staging tile (no pool-reuse hazard); the
        # int32->bf16 casts run on DVE per chunk, just before that chunk's
        # one-hot ops, so the DVE queue never head-of-line blocks.
        goffs = [0]
        for fgg in gs:
            goffs.append(goffs[-1] + fgg)
        slab32s = []
        slabhs = []  # per group: (tile, unit_start)

        # Labels in 4 big chunks (4KB runs) on the ACT HW-DGE ring, issued
        # up front — their transfers overlap the pred stream on the sync
        # ring, and the ACT casts queue up behind the 4 issue instructions.
        lch = max(ftot // 4, 1)
        for i in range(ftot // lch):
            off = i * lch
            nc.scalar.dma_start(
                lab32[:, off : off + lch], lab_ext[:, off : off + lch]
            )

        def unit_map_entry(u):
            # group containing unit u, and unit index within that group
            t = u * QB
            g = 0
            while goffs[g + 1] <= t:
                g += 1
            return slabhs[g][0], (t - goffs[g]) // QB

        def emit_cast(g):
            fgg = gs[g]
            s32 = slab32s[g]
            # slabh layout: [p, (u, b, c)] with c in 0..8 (8 pred channels +
            # ones slot) — each unit's stationary [128, 36] is a contiguous
            # slice, and the pred cast is a fully DENSE copy (runs of 8)
            # matching the host's channel-interleaved layout: strided ACT
            # reads measured ~2.2 cyc/elem vs 1 dense.
            slabh = slabh_bufs[g % len(slabh_bufs)]
            slabh_r = slabh[:, : NSTAT * fgg // QB].rearrange(
                "p (u b c) -> p u b c", c=NCH, b=QB
            )
            slab32_r = s32.rearrange("p (u b c) -> p u b c", c=C, b=QB)
            nc.scalar.activation(
                slabh_r[:, :, :, :C],
                slab32_r,
                mybir.ActivationFunctionType.Copy,
                scale=PRED_SCALE,
            )
            slabhs.append((slabh, goffs[g] // QB))

        NSLABH = len(slabh_bufs)

        # ---- pipelined master loop over one-hot chunks ---------------------
        # Emission order IS dependency-discovery order for the online Tile
        # tracker, so each chunk iteration emits: casts for its groups ->
        # pred DMA issues (2-group lookahead) -> labels copy + one-hot ->
        # matmuls.  slab32 pool reuse (bufs=4) and slabh rotation (NSLABH
        # static bufs) both stay within the already-emitted-reader window.
        next_dma = 0
        next_cast = 0
        u = 0
        coff = 0

        def emit_dma(g):
            goff = goffs[g]
            fgg = gs[g]
            slab32 = slab32_pool.tile([128, C * fg], f32, tag="slab32")
            s32 = slab32[:, : C * fgg]
            nc.sync.dma_start(s32, pred_ext[:, goff * C : (goff + fgg) * C])
            slab32s.append(s32)

        for ci, fcc in enumerate(cs):
            while next_cast < len(gs) and goffs[next_cast] < coff + fcc:
                while next_dma < len(gs) and (
                    goffs[next_dma] < coff + fcc + 2 * fg
                    and next_dma < next_cast + 4
                ):
                    emit_dma(next_dma)
                    next_dma += 1
                emit_cast(next_cast)
                next_cast += 1
            nc.vector.tensor_copy(
                labbf[:, coff : coff + fcc], lab32[:, coff : coff + fcc]
            )
            oh = oh_pool.tile([128, K * fc], fp16, tag="oh")
            oh_r = oh[:, : K * fcc].rearrange("p (u j b) -> p u j b", j=K, b=QB)
            in0 = labbf[:, coff : coff + fcc].rearrange("p (u b) -> p u b", b=QB)
            for j in range(K):
                nc.vector.tensor_scalar(
                    oh_r[:, :, j, :],
                    in0,
                    float(j + 1),
                    None,
                    mybir.AluOpType.is_equal,
                )
            for uc in range(fcc // QB):
                sh, ug = unit_map_entry(u)
                t = u % 2
                nc.tensor.matmul(
                    psum_s[64 * t : 64 * t + NSTAT, :],
                    sh[:, ug * NSTAT : (ug + 1) * NSTAT],
                    oh[:, uc * NMOV : (uc + 1) * NMOV],
                    start=(u < 2),
                    stop=(u >= nunits - 2),
                    tile_position=(0, 64 * t),
                    skip_group_check=True,
                )
                u += 1
            coff += fcc

        # ---- output --------------------------------------------------------
        outt = out_pool.tile([128, NMOV], f32)
        nc.vector.memset(outt[:], 0.0)
        nc.vector.tensor_copy(outt[:NSTAT, :], psum_s[:NSTAT, :])
        nc.vector.tensor_copy(
            outt[64 : 64 + NSTAT, :], psum_s[64 : 64 + NSTAT, :]
        )
        if warm:
            nc.vector.tensor_copy(outaux[32:33, :], warm_ps[32:33, :])
        nc.sync.dma_start(out_ext[:], outt[:])
        nc.sync.dma_start(aux_ext[:], outaux[:40, :])
    nc.compile()
    return nc


@functools.lru_cache(maxsize=1)
def _get_program():
    return build_nc()


def pack_core(pred_core, labels_core, pcore=PCORE):
    """Host-side packing into the kernel's pixel-major DMA layouts."""
    ftot = pcore // 128
    pred_r = np.ascontiguousarray(
        pred_core.reshape(C, 128, ftot).transpose(1, 2, 0)
    ).reshape(128, ftot * C)
    lab_r = labels_core.reshape(128, ftot)
    return pred_r, lab_r


def make_in_maps(pred_flat, labels_flat, pcore=PCORE, ncores=NCORES):
    in_maps = []
    for i in range(ncores):
        sl = slice(i * pcore, (i + 1) * pcore)
        pred_r, lab_r = pack_core(pred_flat[:, sl], labels_flat[sl], pcore)
        in_maps.append({"pred": pred_r, "labels": lab_r})
    return in_maps


def extract_SN(res_core):
    """From one core's outputs: S_scaled [C, K] and N [K]."""
    ps = res_core["out_s"].astype(np.float64)  # [128, NMOV]
    S = np.zeros((C, K))
    N = np.zeros(K)
    for t in range(2):
        r = ps[64 * t : 64 * t + NSTAT, :].reshape(QB, NCH, K, QB)
        d = r[np.arange(QB), :, :, np.arange(QB)].sum(axis=0)  # [NCH, K]
        S += d[:C, :]
        N += d[C, :]
    return S, N


def finish_host(results, num_kernel):
    S = np.zeros((C, K))
    N = np.zeros(K)
    for r in results:
        Si, Ni = extract_SN(r)
        S += Si
        N += Ni
    S /= PRED_SCALE
    A = N * np.sum(S * S, axis=0)  # [K]
    kk = int(num_kernel)
    A = A[:kk]
    pair = A[:, None] + A[None, :]
    Dm = np.maximum(SIGMA_DIS - np.sqrt(pair), 0.0)
    term = np.log(Dm * Dm + 1.0)
    L = float(np.sum(np.triu(term, k=1)))
    L *= (kk - 1) / kk
    return np.float32(L)


_last_results = None


def kernel(pred_similarities, regions_mask, kernel_labels, num_kernel, **kw):
    global _last_results
    from concourse.bass_utils import run_bass_kernel_spmd

    pred_flat = np.asarray(pred_similarities, dtype=np.float32).reshape(C, PTOT)
    labels_flat = np.asarray(kernel_labels, dtype=np.int32).reshape(PTOT)

    nc = _get_program()
    in_maps = make_in_maps(pred_flat, labels_flat)
    res = run_bass_kernel_spmd(nc, in_maps, list(range(NCORES)))
    _last_results = res
    return finish_host(
        [res.results[i] for i in range(NCORES)], num_kernel
    )

